# revision 13
# baseline (speedup 1.0000x reference)
"""Trainium2 Bass kernel for nn_AttentionModule_24068996726850.

Mathematical collapse: the reference expands [N, C] -> [N, C, L] with L
identical columns, so every [*, *, l] slice is identical.  The softmax
over L of constant logits is uniform (1/L), and sum_l attn*value
reduces to `value` itself.  The whole module is therefore:

    P   = relu(features  @ Wk.T)
    Hh  = relu(bn1(features2 @ Wv1.T))
    V   = relu(bn2(Hh @ Wv2.T))
    Cc  = sigmoid(P @ Wvc.T)
    out = V + P * Cc          # [N, 512]

(verified to ~7e-7 scale-relative against the full reference formula).
Wa / Wqk / key_e drop out entirely.

Device dataflow (per core, pure data-parallel over N):
  - activations kept transposed [512 channels x n] so the channel axis
    sits on SBUF partitions; BN folds into per-partition scale/bias of a
    single ACT activation instruction; no on-chip transposes anywhere.
  - weights pre-transposed on host to [cin, cout] = lhsT layout.
  - 4 chained matmuls per 512-column chunk, float32r (full-rate fp32).
"""

import numpy as np
from contextlib import ExitStack

N_CORES = 8
N_TOTAL = 65536
C = 512            # input channels
D = 512            # output channels
NS = N_TOTAL // N_CORES   # 8192 rows per core
NF = 512           # columns (rows of the original problem) per chunk
NCHUNK = NS // NF  # 16
KG = C // 128      # 4 contraction partition-groups
MG = D // 128      # 4 output-channel partition-groups
BN_EPS = 1e-5

MM_DTYPE = "bfloat16"   # matmul input dtype: float32r | float32 | bfloat16

_CACHE = {}


def _patch_tail_drain():
    """The kernel-tail drain emitted at TileContext exit carries one wait
    per logical proc (13 here) — far over walrus's one-sync-wait budget.
    Split it into per-proc drains with identical semantics."""
    import concourse.tile as tile
    from concourse.vector_clock import ScopedClock, VectorClock

    if getattr(tile.TileContext, "_tail_split_patched", False):
        return

    def _split(self, tick_clock, wait_clock):
        gc = tick_clock.global_clock
        n = len(gc)
        for p in range(n):
            t = gc[p]
            if t <= 0:
                continue
            vec = [0] * n
            vec[p] = t
            d = self.nc.sync.drain()
            wait_clock.add_sem_waits(
                d.ins, ScopedClock({None: VectorClock(vec)})
            )
        self.nc.all_engine_barrier()
        assert self.sems is not None
        popped = self.nc._tile_sem_poison_stack.pop()
        assert popped is self._sem_poison
        self.nc.clear_and_free_semaphores(
            list(self.sems.allocated().values()))
        self.nc.all_engine_barrier()

    tile.TileContext._drain_and_barrier = _split
    tile.TileContext._tail_split_patched = True


def _build_program():
    import concourse.bass as bass
    import concourse.mybir as mybir
    import concourse.tile as tile
    from concourse.bass import ds
    from concourse.tile import add_dep_helper

    FP32 = mybir.dt.float32
    MMDT = getattr(mybir.dt, MM_DTYPE)
    AF = mybir.ActivationFunctionType

    # HARD CONSTRAINT on this toolchain: walrus allows at most ONE
    # sync-wait per instruction.  Tile elides a wait only if this engine
    # already waited a >= tick of that semaphore via an earlier
    # DATA-dependent instruction (manual sync edges / Drains don't
    # count).  The kernel threads tiny observer ops through every engine
    # so each real instruction needs <= 1 wait:
    #   PE  <- ACT ticks: tiny matmuls reading evicted tiles
    #   ACT <- DVE ticks: tiny Copy activations
    #   DVE <- PE ticks:  write-once PSUM "mailbox" stamped by PE
    #   SP  <- DVE ticks: tiny SBUF->DRAM observer DMA
    # Activations bounce DRAM -> per-k landing tile -> DVE copy ->
    # compute tile so a landing slot's accessor set is {one DMA, one DVE
    # copy} and its reuse wait collapses to one DMA-queue semaphore.
    # All DRAM tensors are chunk-major so per-chunk accesses are
    # disjoint regions (no conservative overlap waits).

    _patch_tail_drain()
    nc = bass.Bass()

    ft = nc.declare_dram_parameter("ft", [NCHUNK, 128, KG * NF], MMDT,
                                   isOutput=False)
    f2t = nc.declare_dram_parameter("f2t", [NCHUNK, 128, KG * NF], MMDT,
                                    isOutput=False)
    wk = nc.declare_dram_parameter("wk", [C, D], MMDT, isOutput=False)
    wv1 = nc.declare_dram_parameter("wv1", [C, D], MMDT, isOutput=False)
    wv2 = nc.declare_dram_parameter("wv2", [D, D], MMDT, isOutput=False)
    wvc = nc.declare_dram_parameter("wvc", [D, D], MMDT, isOutput=False)
    # [bn1s | bn1b | bn2s | bn2b | zeros] in one tensor -> one DMA
    bnv = nc.declare_dram_parameter("bnv", [128, 5 * MG + 128], FP32,
                                    isOutput=False)
    # one output tensor per chunk: DRAM dep tracking is tensor-granular,
    # so a shared output tensor would chain every out-DMA via WAW waits
    outs = [
        nc.declare_dram_parameter(f"out{j}", [D, NF], MMDT, isOutput=True)
        for j in range(NCHUNK)
    ]

    with tile.TileContext(nc) as tc:
        with ExitStack() as ctx:
            consts = ctx.enter_context(tc.tile_pool(name="consts", bufs=1))

            def load_weight(dram):
                t = consts.tile([128, KG, D], MMDT, tag=f"w_{dram.name}")
                nc.scalar.dma_start(
                    t[:], dram[:].rearrange("(k p) o -> p k o", p=128))
                return t

            wk_t = load_weight(wk)

            # weights + bnv issue on the ACT HWDGE ring; activation
            # chunks issue on the (otherwise idle) SP HWDGE ring so the
            # two rings fill in parallel at startup and the per-chunk
            # load issue never contends with ACT eviction work
            io_pool_early = ctx.enter_context(tc.tile_pool(name="ioe", bufs=2))
            early_loads = {}
            HLF = KG * NF // 2

            def early_load(j0):
                ftX0 = io_pool_early.tile([128, KG * NF], MMDT, tag="ftX",
                                          name=f"ftXe_{j0}")
                f2tX0 = io_pool_early.tile([128, KG * NF], MMDT, tag="f2tX",
                                           name=f"f2tXe_{j0}")
                nc.scalar.dma_start(ftX0[:, 0:HLF], ft[j0, :, 0:HLF])
                nc.scalar.dma_start(ftX0[:, HLF:], ft[j0, :, HLF:])
                nc.scalar.dma_start(f2tX0[:, 0:HLF], f2t[j0, :, 0:HLF])
                nc.scalar.dma_start(f2tX0[:, HLF:], f2t[j0, :, HLF:])
                early_loads[j0] = (ftX0, f2tX0)

            early_load(0)
            wv1_t = load_weight(wv1)
            wvc_t = load_weight(wvc)
            wv2_t = load_weight(wv2)

            bnv_t = consts.tile([128, 5 * MG + 128], FP32, tag="bnv")
            nc.scalar.dma_start(bnv_t[:], bnv[:])
            bn1s_t = bnv_t[:, 0 * MG : 1 * MG]
            bn1b_t = bnv_t[:, 1 * MG : 2 * MG]
            bn2s_t = bnv_t[:, 2 * MG : 3 * MG]
            bn2b_t = bnv_t[:, 3 * MG : 4 * MG]
            zero_t = bnv_t[:, 4 * MG : 5 * MG]
            ident_t = bnv_t[:, 5 * MG : 5 * MG + 128]

            early_load(1)

            # write-once scratch columns for observer writes
            scrA = consts.tile([128, 3 * NCHUNK + 8], FP32, tag="scrA")
            scrD = consts.tile([128, 2 * NCHUNK + 16], FP32, tag="scrD")

            io_pool = ctx.enter_context(tc.tile_pool(name="io", bufs=2))
            act_pool = ctx.enter_context(tc.tile_pool(name="acts", bufs=2))
            psA = ctx.enter_context(tc.tile_pool(name="psA", bufs=4, space="PSUM"))
            psB = ctx.enter_context(tc.tile_pool(name="psB", bufs=3, space="PSUM"))
            psM = ctx.enter_context(tc.tile_pool(name="psM", bufs=1, space="PSUM"))
            # weight-observer target (write-once columns)
            mb = psM.tile([128, 8], FP32, tag="mb")

            ALU = mybir.AluOpType

            def mm_stage(pool, w_t, src_t, dst_t, mo, func,
                         scale=1.0, bias=0.0, after=None, act_after=None,
                         dve_evict=False):
                ps = pool.tile([128, NF], FP32, tag="ps")
                last = None
                for k in range(KG):
                    mm = nc.tensor.matmul(
                        ps[:],
                        lhsT=w_t[:, k, ds(mo * 128, 128)],
                        rhs=src_t[:, ds(k * NF, NF)],
                        start=(k == 0),
                        stop=(k == KG - 1),
                    )
                    last = mm.ins
                    if after is not None:
                        add_dep_helper(mm.ins, after, sync=False,
                                       reason="mm order")
                dst = dst_t[:, ds(mo * NF, NF)]
                if dve_evict:
                    # relu (+ bias) eviction on the DVE to offload the
                    # saturated ACT engine; BN scale is folded into the
                    # weights host-side
                    if isinstance(bias, float):
                        act = nc.vector.tensor_scalar_max(dst, ps[:], 0.0)
                    else:
                        act = nc.vector.tensor_scalar(
                            dst, ps[:], bias, 0.0, ALU.add, ALU.max)
                else:
                    act = nc.scalar.activation(
                        dst, ps[:], func, scale=scale, bias=bias)
                if act_after is not None:
                    add_dep_helper(act.ins, act_after, sync=False,
                                   reason="act order")
                return last, act.ins

            state = {}

            for j in range(NCHUNK):
                s = {}
                s2 = state.get(j - 2, {})
                s1 = state.get(j - 1, {})

                # ---- loads: two half-DMAs per tensor ----
                if j < 2:
                    ftX, f2tX = early_loads[j]
                    lds = ()
                else:
                    ftX = io_pool_early.tile([128, KG * NF], MMDT, tag="ftX",
                                             name=f"ftX_{j}")
                    f2tX = io_pool_early.tile([128, KG * NF], MMDT,
                                              tag="f2tX", name=f"f2tX_{j}")
                    lds = (
                        nc.scalar.dma_start(ftX[:, 0:HLF], ft[j, :, 0:HLF]),
                        nc.scalar.dma_start(ftX[:, HLF:], ft[j, :, HLF:]),
                        nc.scalar.dma_start(f2tX[:, 0:HLF], f2t[j, :, 0:HLF]),
                        nc.scalar.dma_start(f2tX[:, HLF:], f2t[j, :, HLF:]),
                    )
                for d in lds:
                    if "ao_last" in s1:
                        add_dep_helper(d.ins, s1["ao_last"], sync=False,
                                       reason="loads after prev ao")

                p_t = act_pool.tile([128, MG * NF], MMDT, tag="P")
                h_t = act_pool.tile([128, MG * NF], MMDT, tag="H")
                v_t = act_pool.tile([128, MG * NF], MMDT, tag="V")
                c_t = act_pool.tile([128, MG * NF], MMDT, tag="Cc")
                o_t = io_pool.tile([128, MG * NF], MMDT, tag="O")

                # ---- ACT observers ----
                if j == 0:
                    ao_last = nc.scalar.activation(
                        scrA[0:1, 0:1], bnv_t[0:1, 0:1], AF.Copy).ins
                else:
                    ao1 = nc.scalar.activation(
                        scrA[0:1, ds(3 * j, 1)],
                        s1["o_t"][0:1, 0:1], AF.Copy).ins
                    ao2 = nc.scalar.activation(
                        scrA[0:1, ds(3 * j + 1, 1)],
                        s1["v_t"][0:1, ds(3 * NF, 1)], AF.Copy).ins
                    add_dep_helper(ao2, ao1, sync=False, reason="act obs order")
                    ao_last = ao2
                s["ao_last"] = ao_last

                # ---- PE observers (interleaved with stages below) ----
                # j=0: one tiny matmul per weight, placed just before the
                # stage that uses it, so the P stream starts as soon as
                # wk+chunk0 land instead of after ALL weights.
                # j>0: obsA (ACT ticks thru C3 evict of j-1) before P;
                # obsB (ACT ticks thru V3 evict of j-1) after P, before H
                # — V3's eviction only completes ~0.7us after the last
                # j-1 matmul, so observing it at chunk start stalls PE.
                def weight_obs(i, w_t, after):
                    ob = nc.tensor.matmul(
                        mb[:, ds(2 * i, 2)],
                        lhsT=w_t[:, 0, 0:128],
                        rhs=w_t[:, 0, 0:2], start=True, stop=True).ins
                    if after is not None:
                        add_dep_helper(ob, after, sync=False,
                                       reason="wobs order")
                    return ob

                obs_c = obs_v = None  # j=0 weight observers, emitted below
                if j == 0:
                    obs_p = weight_obs(0, wk_t, None)
                else:
                    ps0 = psA.tile([128, NF], FP32, tag="ps")
                    obs_p = nc.tensor.matmul(
                        ps0[0:128, 0:2],
                        lhsT=s1["c_t"][:, ds(3 * NF, 128)],
                        rhs=s1["c_t"][:, ds(3 * NF, 2)],
                        start=True, stop=True).ins

                # ---- stages ----
                dve_anchor = s2.get("comb_add")
                if j == 0:
                    # one-time: DVE observes the bnv DMA queue before the
                    # bias-carrying evictions read it
                    dve_anchor = nc.vector.tensor_copy(
                        scrD[0:1, ds(2 * NCHUNK + 1, 1)],
                        bnv_t[0:1, 0:1]).ins
                p_last = None
                for mo in range(MG):
                    mm, act = mm_stage(psA, wk_t, ftX, p_t, mo, AF.Relu,
                                       after=obs_p, act_after=dve_anchor,
                                       dve_evict=True)
                    p_last = mm

                if j == 0:
                    obs_h = weight_obs(1, wv1_t, p_last)
                else:
                    ps0 = psB.tile([128, NF], FP32, tag="ps")
                    obs_h = nc.tensor.matmul(
                        ps0[0:128, 0:2],
                        lhsT=s1["v_t"][:, ds(3 * NF, 128)],
                        rhs=s1["v_t"][:, ds(3 * NF, 2)],
                        start=True, stop=True).ins
                    add_dep_helper(obs_h, p_last, sync=False,
                                   reason="obs order")

                h_last = None
                for mo in range(MG):
                    mm, act = mm_stage(
                        psB, wv1_t, f2tX, h_t, mo, AF.Relu,
                        bias=bn1b_t[:, mo : mo + 1],
                        after=obs_h, act_after=dve_anchor,
                        dve_evict=True,
                    )
                    h_last = mm

                if j == 0:
                    obs_c = weight_obs(2, wvc_t, h_last)
                c_last = None
                for mo in range(MG):
                    mm, act = mm_stage(psA, wvc_t, p_t, c_t, mo, AF.Sigmoid,
                                       bias=zero_t[:, mo : mo + 1],
                                       after=obs_c, act_after=ao_last)
                    c_last = mm
                if j == 0:
                    obs_v = weight_obs(3, wv2_t, c_last)
                for mo in range(MG):
                    mm, act = mm_stage(
                        psB, wv2_t, h_t, v_t, mo, AF.Relu,
                        bias=bn2b_t[:, mo : mo + 1],
                        after=obs_v, act_after=ao_last,
                    )

                # ---- combine (DVE) ----
                do1 = None
                if "out_dma" in s2:
                    do1 = nc.vector.tensor_copy(
                        o_t[0:1, 0:1], scrD[0:1, ds(j - 1, 1)]).ins
                    if "comb_add" in s1:
                        add_dep_helper(do1, s1["comb_add"], sync=False,
                                       reason="do1 after j-1 combine")
                # combine runs uniform-MMDT (bf16: 2x DVE rate, half the
                # out-DMA bytes); bitcast only for the 4-byte float32r path
                p_f32 = p_t[:].bitcast(FP32) if MM_DTYPE == "float32r" else p_t[:]
                out_view = outs[j][:].rearrange("(m p) n -> p m n", p=128)
                if j == NCHUNK - 1:
                    # pipeline the final chunk per mo-group so the tail
                    # (evict -> combine -> out-DMA) overlaps instead of
                    # serializing after the last matmul
                    prev_op = do1
                    for mo in range(MG):
                        sl = ds(mo * NF, NF)
                        oc = nc.vector.tensor_copy(
                            scrD[0:1, ds(2 * NCHUNK + 2 + 2 * mo, 1)],
                            c_t[0:1, ds(mo * NF, 1)]).ins
                        if prev_op is not None:
                            add_dep_helper(oc, prev_op, sync=False,
                                           reason="tail order")
                        cm = nc.vector.tensor_mul(
                            o_t[:, sl], p_f32[:, sl], c_t[:, sl])
                        add_dep_helper(cm.ins, oc, sync=False,
                                       reason="tail order")
                        ov = nc.vector.tensor_copy(
                            scrD[0:1, ds(2 * NCHUNK + 3 + 2 * mo, 1)],
                            v_t[0:1, ds(mo * NF, 1)]).ins
                        add_dep_helper(ov, cm.ins, sync=False,
                                       reason="tail order")
                        ca = nc.vector.tensor_add(
                            o_t[:, sl], o_t[:, sl], v_t[:, sl])
                        add_dep_helper(ca.ins, ov, sync=False,
                                       reason="tail order")
                        ao3 = nc.scalar.activation(
                            scrA[0:1, ds(3 * j + 2, 1)] if mo == 0 else
                            scrA[0:1, ds(3 * NCHUNK + mo, 1)],
                            o_t[0:1, ds(mo * NF + 1, 1)], AF.Copy).ins
                        od = nc.scalar.dma_start(out_view[:, mo, :],
                                                 o_t[:, sl])
                        add_dep_helper(od.ins, ao3, sync=False,
                                       reason="od after ao3")
                        prev_op = ca.ins
                    s["comb_add"] = prev_op
                    s["out_dma"] = od.ins
                else:
                    # DVE observes C3's ACT tick, multiplies, then observes
                    # V3's tick and adds — the mul doesn't idle until the
                    # V evictions land
                    do2a = nc.vector.tensor_copy(
                        scrD[0:1, ds(j, 1)], c_t[0:1, ds(3 * NF, 1)]).ins
                    if do1 is not None:
                        add_dep_helper(do2a, do1, sync=False,
                                       reason="dve order")
                    cm = nc.vector.tensor_mul(o_t[:], p_f32, c_t[:])
                    add_dep_helper(cm.ins, do2a, sync=False,
                                   reason="mul order")
                    do2b = nc.vector.tensor_copy(
                        scrD[0:1, ds(NCHUNK + 1 + j, 1)],
                        v_t[0:1, ds(3 * NF, 1)]).ins
                    add_dep_helper(do2b, cm.ins, sync=False,
                                   reason="dve order")
                    ca = nc.vector.tensor_add(o_t[:], o_t[:], v_t[:])
                    add_dep_helper(ca.ins, do2b, sync=False,
                                   reason="add order")
                    s["comb_add"] = ca.ins

                    # ACT observes this chunk's combine so the (ACT-issued)
                    # out-DMA needs no fresh DVE wait of its own
                    ao3 = nc.scalar.activation(
                        scrA[0:1, ds(3 * j + 2, 1)], o_t[0:1, 1:2],
                        AF.Copy).ins
                    od = nc.scalar.dma_start(
                        out_view,
                        o_t[:].rearrange("p (m n) -> p m n", m=MG),
                    )
                    add_dep_helper(od.ins, ao3, sync=False,
                                   reason="od after ao3")
                    s["out_dma"] = od.ins
                s["c_t"] = c_t
                s["v_t"] = v_t
                s["o_t"] = o_t
                state[j] = s

    return nc


def _np_mm_dtype():
    if MM_DTYPE == "bfloat16":
        import ml_dtypes

        return np.dtype(ml_dtypes.bfloat16)
    return np.dtype(np.float32)


def _prep_inputs(inputs):
    mmdt = _np_mm_dtype()
    f = np.asarray(inputs["features"], np.float32)
    f2 = np.asarray(inputs["features2"], np.float32)

    def wT(name):
        return np.ascontiguousarray(np.asarray(inputs[name], np.float32).T)

    wk_h, wv1_h, wv2_h, wvc_h = wT("Wk"), wT("Wv1"), wT("Wv2"), wT("Wvc")

    def bn_inv(pre):
        g = np.asarray(inputs[f"{pre}_gamma"], np.float32)
        v = np.asarray(inputs[f"{pre}_var"], np.float32)
        return g / np.sqrt(v + BN_EPS)

    # fold the BN scale into the weight columns (bias stays separate)
    wv1_h = np.ascontiguousarray(wv1_h * bn_inv("bn1")[None, :])
    wv2_h = np.ascontiguousarray(wv2_h * bn_inv("bn2")[None, :])

    def bn_fold(pre):
        g = np.asarray(inputs[f"{pre}_gamma"], np.float32)
        b = np.asarray(inputs[f"{pre}_beta"], np.float32)
        m = np.asarray(inputs[f"{pre}_mean"], np.float32)
        v = np.asarray(inputs[f"{pre}_var"], np.float32)
        inv = g / np.sqrt(v + BN_EPS)
        shift = b - m * inv
        to_tile = lambda x: np.ascontiguousarray(x.reshape(MG, 128).T)
        return to_tile(inv), to_tile(shift)

    bn1s_h, bn1b_h = bn_fold("bn1")
    bn2s_h, bn2b_h = bn_fold("bn2")
    bnv_h = np.ascontiguousarray(np.concatenate(
        [bn1s_h, bn1b_h, bn2s_h, bn2b_h,
         np.zeros((128, MG), np.float32),
         np.zeros((128, 128), np.float32)],
        axis=1,
    ))

    shared = {
        "wk": wk_h.astype(mmdt), "wv1": wv1_h.astype(mmdt),
        "wv2": wv2_h.astype(mmdt), "wvc": wvc_h.astype(mmdt),
        "bnv": bnv_h,
    }

    def chunked_T(x):  # [NS, C] rows -> [NCHUNK, 128, KG*NF] k-major
        t = x.T.astype(mmdt)  # [C, NS]; c = k*128 + p
        a = t.reshape(KG, 128, NCHUNK, NF)
        return np.ascontiguousarray(
            a.transpose(2, 1, 0, 3).reshape(NCHUNK, 128, KG * NF))

    in_maps = []
    for i in range(N_CORES):
        rows = slice(i * NS, (i + 1) * NS)
        in_maps.append({
            "ft": chunked_T(f[rows]),
            "f2t": chunked_T(f2[rows]),
            **shared,
        })
    return in_maps


def _gather_out(res_map):
    """[NCHUNK x [D, NF]] per-core outputs -> [NS, D] rows."""
    chunks = [np.asarray(res_map[f"out{j}"]) for j in range(NCHUNK)]
    stacked = np.stack(chunks, axis=0)          # [NCHUNK, D, NF]
    return stacked.transpose(0, 2, 1).reshape(NS, D)


def run(inputs, trace=False):
    from concourse.bass_utils import run_bass_kernel_spmd

    if "nc" not in _CACHE:
        _CACHE["nc"] = _build_program()
    nc = _CACHE["nc"]

    in_maps = _prep_inputs(inputs)
    res = run_bass_kernel_spmd(
        nc, in_maps, list(range(N_CORES)), trace=trace
    )
    full = np.concatenate(
        [_gather_out(r) for r in res.results], axis=0
    ).astype(np.float32)
    return full, res


def kernel(**inputs) -> np.ndarray:
    out, _ = run(inputs, trace=False)
    return out



# revision 25
# speedup vs baseline: 1.2962x; 1.2962x over previous
"""Trainium2 Bass kernel for nn_AttentionModule_24068996726850.

Mathematical collapse: the reference expands [N, C] -> [N, C, L] with L
identical columns, so every [*, *, l] slice is identical.  The softmax
over L of constant logits is uniform (1/L), and sum_l attn*value
reduces to `value` itself.  The whole module is therefore:

    P   = relu(features  @ Wk.T)
    Hh  = relu(bn1(features2 @ Wv1.T))
    V   = relu(bn2(Hh @ Wv2.T))
    Cc  = sigmoid(P @ Wvc.T)
    out = V + P * Cc          # [N, 512]

(verified to ~7e-7 scale-relative against the full reference formula).
Wa / Wqk / key_e drop out entirely.

Device dataflow (per core, pure data-parallel over N):
  - activations kept transposed [512 channels x n] so the channel axis
    sits on SBUF partitions; BN folds into per-partition scale/bias of a
    single ACT activation instruction; no on-chip transposes anywhere.
  - weights pre-transposed on host to [cin, cout] = lhsT layout.
  - 4 chained matmuls per 512-column chunk, float32r (full-rate fp32).
"""

import numpy as np
from contextlib import ExitStack

N_CORES = 8
N_TOTAL = 65536
C = 512            # input channels
D = 512            # output channels
NS = N_TOTAL // N_CORES   # 8192 rows per core
NF = 512           # columns (rows of the original problem) per chunk
NCHUNK = NS // NF  # 16
KG = C // 128      # 4 contraction partition-groups
MG = D // 128      # 4 output-channel partition-groups
BN_EPS = 1e-5

MM_DTYPE = "bfloat16"   # matmul input dtype: float32r | float32 | bfloat16

_CACHE = {}


def _patch_tail_drain():
    """The kernel-tail drain emitted at TileContext exit carries one wait
    per logical proc (13 here) — far over walrus's one-sync-wait budget.
    Split it into per-proc drains with identical semantics."""
    import concourse.tile as tile
    from concourse.vector_clock import ScopedClock, VectorClock

    if getattr(tile.TileContext, "_tail_split_patched", False):
        return

    def _split(self, tick_clock, wait_clock):
        gc = tick_clock.global_clock
        n = len(gc)
        for p in range(n):
            t = gc[p]
            if t <= 0:
                continue
            vec = [0] * n
            vec[p] = t
            d = self.nc.sync.drain()
            wait_clock.add_sem_waits(
                d.ins, ScopedClock({None: VectorClock(vec)})
            )
        self.nc.all_engine_barrier()
        assert self.sems is not None
        popped = self.nc._tile_sem_poison_stack.pop()
        assert popped is self._sem_poison
        self.nc.clear_and_free_semaphores(
            list(self.sems.allocated().values()))
        self.nc.all_engine_barrier()

    tile.TileContext._drain_and_barrier = _split
    tile.TileContext._tail_split_patched = True


def _build_program():
    import concourse.bass as bass
    import concourse.mybir as mybir
    import concourse.tile as tile
    from concourse.bass import ds
    from concourse.tile import add_dep_helper

    FP32 = mybir.dt.float32
    MMDT = getattr(mybir.dt, MM_DTYPE)
    AF = mybir.ActivationFunctionType

    # HARD CONSTRAINT on this toolchain: walrus allows at most ONE
    # sync-wait per instruction.  Tile elides a wait only if this engine
    # already waited a >= tick of that semaphore via an earlier
    # DATA-dependent instruction (manual sync edges / Drains don't
    # count).  The kernel threads tiny observer ops through every engine
    # so each real instruction needs <= 1 wait:
    #   PE  <- ACT ticks: tiny matmuls reading evicted tiles
    #   ACT <- DVE ticks: tiny Copy activations
    #   DVE <- PE ticks:  write-once PSUM "mailbox" stamped by PE
    #   SP  <- DVE ticks: tiny SBUF->DRAM observer DMA
    # Activations bounce DRAM -> per-k landing tile -> DVE copy ->
    # compute tile so a landing slot's accessor set is {one DMA, one DVE
    # copy} and its reuse wait collapses to one DMA-queue semaphore.
    # All DRAM tensors are chunk-major so per-chunk accesses are
    # disjoint regions (no conservative overlap waits).

    _patch_tail_drain()
    nc = bass.Bass()

    # ft|f2t merged chunk-major: ONE load DMA per chunk (issue cost on the
    # ACT HWDGE ring is ~0.7us per dma_start, so fewer+larger wins)
    fx = nc.declare_dram_parameter("fx", [NCHUNK, 128, 2 * KG * NF], MMDT,
                                   isOutput=False)
    wk = nc.declare_dram_parameter("wk", [C, D], MMDT, isOutput=False)
    wv1 = nc.declare_dram_parameter("wv1", [C, D], MMDT, isOutput=False)
    wv2 = nc.declare_dram_parameter("wv2", [D, D], MMDT, isOutput=False)
    # C-stage runs fp8e4 DoubleRow (2 contraction rows/cycle): the only
    # stage where fp8 quantization is safe (sigmoid damps the error and
    # the result only modulates the P*C term)
    FP8 = mybir.dt.float8e4
    wvc = nc.declare_dram_parameter("wvc", [D, D], FP8, isOutput=False)
    # [bn1s | bn1b | bn2s | bn2b | zeros] in one tensor -> one DMA
    bnv = nc.declare_dram_parameter("bnv", [128, 5 * MG + 128], FP32,
                                    isOutput=False)
    # one output tensor per chunk: DRAM dep tracking is tensor-granular,
    # so a shared output tensor would chain every out-DMA via WAW waits
    outs = [
        nc.declare_dram_parameter(f"out{j}", [D, NF], MMDT, isOutput=True)
        for j in range(NCHUNK)
    ]

    with tile.TileContext(nc) as tc:
        with ExitStack() as ctx:
            consts = ctx.enter_context(tc.tile_pool(name="consts", bufs=1))

            def load_weight(dram, dt=None):
                t = consts.tile([128, KG, D], dt or MMDT, tag=f"w_{dram.name}")
                nc.scalar.dma_start(
                    t[:], dram[:].rearrange("(k p) o -> p k o", p=128))
                return t

            wk_t = load_weight(wk)

            # weights + bnv issue on the ACT HWDGE ring; activation
            # chunks issue on the (otherwise idle) SP HWDGE ring so the
            # two rings fill in parallel at startup and the per-chunk
            # load issue never contends with ACT eviction work
            io_pool_early = ctx.enter_context(tc.tile_pool(name="ioe", bufs=2))
            early_loads = {}
            HLF = KG * NF // 2

            def early_load(j0):
                fxX0 = io_pool_early.tile([128, 2 * KG * NF], MMDT,
                                          tag="fxX", name=f"fxXe_{j0}")
                nc.scalar.dma_start(fxX0[:], fx[j0])
                early_loads[j0] = fxX0

            early_load(0)
            wv1_t = load_weight(wv1)
            wvc_t = load_weight(wvc, FP8)
            wv2_t = load_weight(wv2)

            bnv_t = consts.tile([128, 5 * MG + 128], FP32, tag="bnv")
            nc.scalar.dma_start(bnv_t[:], bnv[:])
            bn1s_t = bnv_t[:, 0 * MG : 1 * MG]
            bn1b_t = bnv_t[:, 1 * MG : 2 * MG]
            bn2s_t = bnv_t[:, 2 * MG : 3 * MG]
            bn2b_t = bnv_t[:, 3 * MG : 4 * MG]
            zero_t = bnv_t[:, 4 * MG : 5 * MG]
            ident_t = bnv_t[:, 5 * MG : 5 * MG + 128]

            early_load(1)

            # write-once scratch columns for observer writes
            scrA = consts.tile([128, 3 * NCHUNK + 16], FP32, tag="scrA")
            scrD = consts.tile([128, 2 * NCHUNK + 16], FP32, tag="scrD")

            io_pool = ctx.enter_context(tc.tile_pool(name="io", bufs=2))
            act_pool = ctx.enter_context(tc.tile_pool(name="acts", bufs=2))
            psA = ctx.enter_context(tc.tile_pool(name="psA", bufs=4, space="PSUM"))
            psB = ctx.enter_context(tc.tile_pool(name="psB", bufs=3, space="PSUM"))
            psM = ctx.enter_context(tc.tile_pool(name="psM", bufs=1, space="PSUM"))
            # weight-observer target (write-once columns)
            mb = psM.tile([128, 8], FP32, tag="mb")

            ALU = mybir.AluOpType

            def mm_stage(pool, w_t, src_t, dst_t, mo, func,
                         scale=1.0, bias=0.0, after=None, act_after=None,
                         dve_evict=False, src_base=0, fp8_dst=None):
                ps = pool.tile([128, NF], FP32, tag="ps")
                last = None
                for k in range(KG):
                    mm = nc.tensor.matmul(
                        ps[:],
                        lhsT=w_t[:, k, ds(mo * 128, 128)],
                        rhs=src_t[:, ds(src_base + k * NF, NF)],
                        start=(k == 0),
                        stop=(k == KG - 1),
                    )
                    last = mm.ins
                    if after is not None:
                        add_dep_helper(mm.ins, after, sync=False,
                                       reason="mm order")
                dst = dst_t[:, ds(mo * NF, NF)]
                if dve_evict:
                    # relu (+ bias) eviction on the DVE to offload the
                    # saturated ACT engine; BN scale is folded into the
                    # weights host-side
                    if isinstance(bias, float):
                        act = nc.vector.tensor_scalar_max(dst, ps[:], 0.0)
                    else:
                        act = nc.vector.tensor_scalar(
                            dst, ps[:], bias, 0.0, ALU.add, ALU.max)
                else:
                    act = nc.scalar.activation(
                        dst, ps[:], func, scale=scale, bias=bias)
                if act_after is not None:
                    add_dep_helper(act.ins, act_after, sync=False,
                                   reason="act order")
                if fp8_dst is not None:
                    # second eviction of the same PSUM: fp8 copy for the
                    # DoubleRow C matmul (P <= ~6 fits e4m3 unscaled)
                    nc.vector.tensor_scalar_max(
                        fp8_dst[:, ds(mo * NF, NF)], ps[:], 0.0)
                return last, act.ins

            WVC_SCALE = 512.0  # host folds *512 into fp8 Wvc; descale here

            def dr_stage(pool, w_t, src_t, dst_t, mo, func,
                         scale=1.0, bias=0.0, after=None, act_after=None):
                # fp8 DoubleRow: 2 contraction-rows/cycle, so KG/2 matmuls
                ps = pool.tile([128, NF], FP32, tag="ps")
                last = None
                for k2 in range(KG // 2):
                    mm = nc.tensor.matmul(
                        ps[:],
                        lhsT=w_t[:, ds(2 * k2, 2), ds(mo * 128, 128)],
                        rhs=src_t[:, ds(2 * k2 * NF, 2 * NF)].rearrange(
                            "p (two n) -> p two n", two=2),
                        start=(k2 == 0),
                        stop=(k2 == KG // 2 - 1),
                        perf_mode=mybir.MatmulPerfMode.DoubleRow,
                    )
                    last = mm.ins
                    if after is not None:
                        add_dep_helper(mm.ins, after, sync=False,
                                       reason="mm order")
                act = nc.scalar.activation(
                    dst_t[:, ds(mo * NF, NF)], ps[:], func,
                    scale=scale, bias=bias)
                if act_after is not None:
                    add_dep_helper(act.ins, act_after, sync=False,
                                   reason="act order")
                return last, act.ins

            state = {}

            for j in range(NCHUNK):
                s = {}
                s2 = state.get(j - 2, {})
                s1 = state.get(j - 1, {})

                # ---- load: ONE merged ft|f2t DMA per chunk ----
                if j < 2:
                    fxX = early_loads[j]
                else:
                    fxX = io_pool_early.tile([128, 2 * KG * NF], MMDT,
                                             tag="fxX", name=f"fxX_{j}")
                    ld = nc.scalar.dma_start(fxX[:], fx[j])
                    if "ao_last" in s1:
                        add_dep_helper(ld.ins, s1["ao_last"], sync=False,
                                       reason="loads after prev ao")
                p_t = act_pool.tile([128, MG * NF], MMDT, tag="P")
                p8_t = act_pool.tile([128, MG * NF], FP8, tag="P8")
                h_t = act_pool.tile([128, MG * NF], MMDT, tag="H")
                v_t = act_pool.tile([128, MG * NF], MMDT, tag="V")
                c_t = act_pool.tile([128, MG * NF], MMDT, tag="Cc")
                o_t = io_pool.tile([128, MG * NF], MMDT, tag="O")

                # ---- ACT observers ----
                if j == 0:
                    ao_last = nc.scalar.activation(
                        scrA[0:1, 0:1], bnv_t[0:1, 0:1], AF.Copy).ins
                    # pre-warm ACT's DMA-lane wait history: the chunk-j
                    # load DMA's round-robin lane reuses a startup DMA's
                    # lane; without an earlier ACT-side wait on that lane
                    # the load would carry 2 sync waits (walrus max 1).
                    # Tiny data-dependent copies give ACT those waits.
                    prewarm = [wk_t, wv1_t, wv2_t]
                    for wi, w_t in enumerate(prewarm):
                        aw = nc.scalar.activation(
                            scrA[0:1, ds(3 * NCHUNK + 8 + wi, 1)],
                            w_t[0:1, 0, 0:1], AF.Copy).ins
                        add_dep_helper(aw, ao_last, sync=False,
                                       reason="prewarm order")
                        ao_last = aw
                    for ei, e_t in enumerate((early_loads[0],
                                              early_loads[1])):
                        aw = nc.scalar.activation(
                            scrA[0:1, ds(3 * NCHUNK + 12 + ei, 1)],
                            e_t[0:1, 0:1], AF.Copy).ins
                        add_dep_helper(aw, ao_last, sync=False,
                                       reason="prewarm order")
                        ao_last = aw
                else:
                    ao1 = nc.scalar.activation(
                        scrA[0:1, ds(3 * j, 1)],
                        s1["o_t"][0:1, 0:1], AF.Copy).ins
                    ao2 = nc.scalar.activation(
                        scrA[0:1, ds(3 * j + 1, 1)],
                        s1["v_t"][0:1, ds(3 * NF, 1)], AF.Copy).ins
                    add_dep_helper(ao2, ao1, sync=False, reason="act obs order")
                    ao_last = ao2
                s["ao_last"] = ao_last

                # ---- PE observers (interleaved with stages below) ----
                # j=0: one tiny matmul per weight, placed just before the
                # stage that uses it, so the P stream starts as soon as
                # wk+chunk0 land instead of after ALL weights.
                # j>0: obsA (ACT ticks thru C3 evict of j-1) before P;
                # obsB (ACT ticks thru V3 evict of j-1) after P, before H
                # — V3's eviction only completes ~0.7us after the last
                # j-1 matmul, so observing it at chunk start stalls PE.
                def weight_obs(i, w_t, after):
                    ob = nc.tensor.matmul(
                        mb[:, ds(2 * i, 2)],
                        lhsT=w_t[:, 0, 0:128],
                        rhs=w_t[:, 0, 0:2], start=True, stop=True).ins
                    if after is not None:
                        add_dep_helper(ob, after, sync=False,
                                       reason="wobs order")
                    return ob

                obs_c = obs_v = None  # j=0 weight observers, emitted below
                if j == 0:
                    obs_p = weight_obs(0, wk_t, None)
                else:
                    ps0 = psA.tile([128, NF], FP32, tag="ps")
                    obs_p = nc.tensor.matmul(
                        ps0[0:128, 0:2],
                        lhsT=s1["c_t"][:, ds(3 * NF, 128)],
                        rhs=s1["c_t"][:, ds(3 * NF, 2)],
                        start=True, stop=True).ins

                # ---- stages ----
                dve_anchor = s2.get("comb_add")
                if j == 0:
                    # one-time: DVE observes the bnv DMA queue before the
                    # bias-carrying evictions read it
                    dve_anchor = nc.vector.tensor_copy(
                        scrD[0:1, ds(2 * NCHUNK + 1, 1)],
                        bnv_t[0:1, 0:1]).ins
                p_last = None
                for mo in range(MG):
                    mm, act = mm_stage(psA, wk_t, fxX, p_t, mo, AF.Relu,
                                       after=obs_p, act_after=dve_anchor,
                                       dve_evict=True, fp8_dst=p8_t)
                    p_last = mm

                if j == 0:
                    obs_h = weight_obs(1, wv1_t, p_last)
                else:
                    ps0 = psB.tile([128, NF], FP32, tag="ps")
                    obs_h = nc.tensor.matmul(
                        ps0[0:128, 0:2],
                        lhsT=s1["v_t"][:, ds(3 * NF, 128)],
                        rhs=s1["v_t"][:, ds(3 * NF, 2)],
                        start=True, stop=True).ins
                    add_dep_helper(obs_h, p_last, sync=False,
                                   reason="obs order")

                h_last = None
                for mo in range(MG):
                    mm, act = mm_stage(
                        psB, wv1_t, fxX, h_t, mo, AF.Relu,
                        bias=bn1b_t[:, mo : mo + 1],
                        after=obs_h, act_after=dve_anchor,
                        dve_evict=True, src_base=KG * NF,
                    )
                    h_last = mm

                if j == 0:
                    obs_c = weight_obs(2, wvc_t, h_last)
                c_last = None
                for mo in range(MG):
                    mm, act = dr_stage(psA, wvc_t, p8_t, c_t, mo, AF.Sigmoid,
                                       scale=1.0 / WVC_SCALE,
                                       bias=zero_t[:, mo : mo + 1],
                                       after=obs_c, act_after=ao_last)
                    c_last = mm
                if j == 0:
                    obs_v = weight_obs(3, wv2_t, c_last)
                for mo in range(MG):
                    mm, act = mm_stage(
                        psB, wv2_t, h_t, v_t, mo, AF.Relu,
                        bias=bn2b_t[:, mo : mo + 1],
                        after=obs_v, act_after=ao_last,
                    )

                # ---- combine (DVE) ----
                do1 = None
                if "out_dma" in s2:
                    do1 = nc.vector.tensor_copy(
                        o_t[0:1, 0:1], scrD[0:1, ds(j - 1, 1)]).ins
                    if "comb_add" in s1:
                        add_dep_helper(do1, s1["comb_add"], sync=False,
                                       reason="do1 after j-1 combine")
                # combine runs uniform-MMDT (bf16: 2x DVE rate, half the
                # out-DMA bytes); bitcast only for the 4-byte float32r path
                p_f32 = p_t[:].bitcast(FP32) if MM_DTYPE == "float32r" else p_t[:]
                out_view = outs[j][:].rearrange("(m p) n -> p m n", p=128)
                if j == NCHUNK - 1:
                    # pipeline the final chunk per mo-group so the tail
                    # (evict -> combine -> out-DMA) overlaps instead of
                    # serializing after the last matmul
                    prev_op = do1
                    for mo in range(MG):
                        sl = ds(mo * NF, NF)
                        oc = nc.vector.tensor_copy(
                            scrD[0:1, ds(2 * NCHUNK + 2 + 2 * mo, 1)],
                            c_t[0:1, ds(mo * NF, 1)]).ins
                        if prev_op is not None:
                            add_dep_helper(oc, prev_op, sync=False,
                                           reason="tail order")
                        cm = nc.vector.tensor_mul(
                            o_t[:, sl], p_f32[:, sl], c_t[:, sl])
                        add_dep_helper(cm.ins, oc, sync=False,
                                       reason="tail order")
                        ov = nc.vector.tensor_copy(
                            scrD[0:1, ds(2 * NCHUNK + 3 + 2 * mo, 1)],
                            v_t[0:1, ds(mo * NF, 1)]).ins
                        add_dep_helper(ov, cm.ins, sync=False,
                                       reason="tail order")
                        ca = nc.vector.tensor_add(
                            o_t[:, sl], o_t[:, sl], v_t[:, sl])
                        add_dep_helper(ca.ins, ov, sync=False,
                                       reason="tail order")
                        ao3 = nc.scalar.activation(
                            scrA[0:1, ds(3 * j + 2, 1)] if mo == 0 else
                            scrA[0:1, ds(3 * NCHUNK + mo, 1)],
                            o_t[0:1, ds(mo * NF + 1, 1)], AF.Copy).ins
                        od = nc.scalar.dma_start(out_view[:, mo, :],
                                                 o_t[:, sl])
                        add_dep_helper(od.ins, ao3, sync=False,
                                       reason="od after ao3")
                        prev_op = ca.ins
                    s["comb_add"] = prev_op
                    s["out_dma"] = od.ins
                else:
                    # DVE observes C3's ACT tick, multiplies, then observes
                    # V3's tick and adds — the mul doesn't idle until the
                    # V evictions land
                    do2a = nc.vector.tensor_copy(
                        scrD[0:1, ds(j, 1)], c_t[0:1, ds(3 * NF, 1)]).ins
                    if do1 is not None:
                        add_dep_helper(do2a, do1, sync=False,
                                       reason="dve order")
                    cm = nc.vector.tensor_mul(o_t[:], p_f32, c_t[:])
                    add_dep_helper(cm.ins, do2a, sync=False,
                                   reason="mul order")
                    do2b = nc.vector.tensor_copy(
                        scrD[0:1, ds(NCHUNK + 1 + j, 1)],
                        v_t[0:1, ds(3 * NF, 1)]).ins
                    add_dep_helper(do2b, cm.ins, sync=False,
                                   reason="dve order")
                    ca = nc.vector.tensor_add(o_t[:], o_t[:], v_t[:])
                    add_dep_helper(ca.ins, do2b, sync=False,
                                   reason="add order")
                    s["comb_add"] = ca.ins

                    # ACT observes this chunk's combine so the (ACT-issued)
                    # out-DMA needs no fresh DVE wait of its own
                    ao3 = nc.scalar.activation(
                        scrA[0:1, ds(3 * j + 2, 1)], o_t[0:1, 1:2],
                        AF.Copy).ins
                    od = nc.scalar.dma_start(
                        out_view,
                        o_t[:].rearrange("p (m n) -> p m n", m=MG),
                    )
                    add_dep_helper(od.ins, ao3, sync=False,
                                   reason="od after ao3")
                    s["out_dma"] = od.ins
                s["c_t"] = c_t
                s["v_t"] = v_t
                s["o_t"] = o_t
                state[j] = s

    return nc


def _np_mm_dtype():
    if MM_DTYPE == "bfloat16":
        import ml_dtypes

        return np.dtype(ml_dtypes.bfloat16)
    return np.dtype(np.float32)


def _prep_inputs(inputs):
    mmdt = _np_mm_dtype()
    f = np.asarray(inputs["features"], np.float32)
    f2 = np.asarray(inputs["features2"], np.float32)

    def wT(name):
        return np.ascontiguousarray(np.asarray(inputs[name], np.float32).T)

    wk_h, wv1_h, wv2_h, wvc_h = wT("Wk"), wT("Wv1"), wT("Wv2"), wT("Wvc")

    def bn_inv(pre):
        g = np.asarray(inputs[f"{pre}_gamma"], np.float32)
        v = np.asarray(inputs[f"{pre}_var"], np.float32)
        return g / np.sqrt(v + BN_EPS)

    # fold the BN scale into the weight columns (bias stays separate)
    wv1_h = np.ascontiguousarray(wv1_h * bn_inv("bn1")[None, :])
    wv2_h = np.ascontiguousarray(wv2_h * bn_inv("bn2")[None, :])

    def bn_fold(pre):
        g = np.asarray(inputs[f"{pre}_gamma"], np.float32)
        b = np.asarray(inputs[f"{pre}_beta"], np.float32)
        m = np.asarray(inputs[f"{pre}_mean"], np.float32)
        v = np.asarray(inputs[f"{pre}_var"], np.float32)
        inv = g / np.sqrt(v + BN_EPS)
        shift = b - m * inv
        to_tile = lambda x: np.ascontiguousarray(x.reshape(MG, 128).T)
        return to_tile(inv), to_tile(shift)

    bn1s_h, bn1b_h = bn_fold("bn1")
    bn2s_h, bn2b_h = bn_fold("bn2")
    bnv_h = np.ascontiguousarray(np.concatenate(
        [bn1s_h, bn1b_h, bn2s_h, bn2b_h,
         np.zeros((128, MG), np.float32),
         np.zeros((128, 128), np.float32)],
        axis=1,
    ))

    import ml_dtypes

    shared = {
        "wk": wk_h.astype(mmdt), "wv1": wv1_h.astype(mmdt),
        "wv2": wv2_h.astype(mmdt),
        # C-stage weights: fp8e4 scaled by 512 (descaled in the sigmoid);
        # |Wvc|*512 tops out ~120, well under e4m3's 240 max
        "wvc": np.ascontiguousarray(
            (wvc_h * 512.0).astype(ml_dtypes.float8_e4m3)),
        "bnv": bnv_h,
    }

    def chunked_T(x):  # [NS, C] rows -> [NCHUNK, 128, KG*NF] k-major
        t = x.T.astype(mmdt)  # [C, NS]; c = k*128 + p
        a = t.reshape(KG, 128, NCHUNK, NF)
        return np.ascontiguousarray(
            a.transpose(2, 1, 0, 3).reshape(NCHUNK, 128, KG * NF))

    in_maps = []
    for i in range(N_CORES):
        rows = slice(i * NS, (i + 1) * NS)
        in_maps.append({
            "fx": np.ascontiguousarray(np.concatenate(
                [chunked_T(f[rows]), chunked_T(f2[rows])], axis=2)),
            **shared,
        })
    return in_maps


def _gather_out(res_map):
    """[NCHUNK x [D, NF]] per-core outputs -> [NS, D] rows."""
    chunks = [np.asarray(res_map[f"out{j}"]) for j in range(NCHUNK)]
    stacked = np.stack(chunks, axis=0)          # [NCHUNK, D, NF]
    return stacked.transpose(0, 2, 1).reshape(NS, D)


def run(inputs, trace=False):
    from concourse.bass_utils import run_bass_kernel_spmd

    if "nc" not in _CACHE:
        _CACHE["nc"] = _build_program()
    nc = _CACHE["nc"]

    in_maps = _prep_inputs(inputs)
    res = run_bass_kernel_spmd(
        nc, in_maps, list(range(N_CORES)), trace=trace
    )
    full = np.concatenate(
        [_gather_out(r) for r in res.results], axis=0
    ).astype(np.float32)
    return full, res


def kernel(**inputs) -> np.ndarray:
    out, _ = run(inputs, trace=False)
    return out



# revision 29
# speedup vs baseline: 1.3056x; 1.0072x over previous
"""Trainium2 Bass kernel for nn_AttentionModule_24068996726850.

Mathematical collapse: the reference expands [N, C] -> [N, C, L] with L
identical columns, so every [*, *, l] slice is identical.  The softmax
over L of constant logits is uniform (1/L), and sum_l attn*value
reduces to `value` itself.  The whole module is therefore:

    P   = relu(features  @ Wk.T)
    Hh  = relu(bn1(features2 @ Wv1.T))
    V   = relu(bn2(Hh @ Wv2.T))
    Cc  = sigmoid(P @ Wvc.T)
    out = V + P * Cc          # [N, 512]

(verified to ~7e-7 scale-relative against the full reference formula).
Wa / Wqk / key_e drop out entirely.

Device dataflow (per core, pure data-parallel over N):
  - activations kept transposed [512 channels x n] so the channel axis
    sits on SBUF partitions; BN folds into per-partition scale/bias of a
    single ACT activation instruction; no on-chip transposes anywhere.
  - weights pre-transposed on host to [cin, cout] = lhsT layout.
  - 4 chained matmuls per 512-column chunk, float32r (full-rate fp32).
"""

import numpy as np
from contextlib import ExitStack

N_CORES = 8
N_TOTAL = 65536
C = 512            # input channels
D = 512            # output channels
NS = N_TOTAL // N_CORES   # 8192 rows per core
NF = 512           # columns (rows of the original problem) per chunk
NCHUNK = NS // NF  # 16
KG = C // 128      # 4 contraction partition-groups
MG = D // 128      # 4 output-channel partition-groups
BN_EPS = 1e-5

MM_DTYPE = "bfloat16"   # matmul input dtype: float32r | float32 | bfloat16

_CACHE = {}


def _patch_tail_drain():
    """The kernel-tail drain emitted at TileContext exit carries one wait
    per logical proc (13 here) — far over walrus's one-sync-wait budget.
    Split it into per-proc drains with identical semantics."""
    import concourse.tile as tile
    from concourse.vector_clock import ScopedClock, VectorClock

    if getattr(tile.TileContext, "_tail_split_patched", False):
        return

    def _split(self, tick_clock, wait_clock):
        gc = tick_clock.global_clock
        n = len(gc)
        for p in range(n):
            t = gc[p]
            if t <= 0:
                continue
            vec = [0] * n
            vec[p] = t
            d = self.nc.sync.drain()
            wait_clock.add_sem_waits(
                d.ins, ScopedClock({None: VectorClock(vec)})
            )
        self.nc.all_engine_barrier()
        assert self.sems is not None
        popped = self.nc._tile_sem_poison_stack.pop()
        assert popped is self._sem_poison
        self.nc.clear_and_free_semaphores(
            list(self.sems.allocated().values()))
        self.nc.all_engine_barrier()

    tile.TileContext._drain_and_barrier = _split
    tile.TileContext._tail_split_patched = True


def _build_program():
    import concourse.bass as bass
    import concourse.mybir as mybir
    import concourse.tile as tile
    from concourse.bass import ds
    from concourse.tile import add_dep_helper

    FP32 = mybir.dt.float32
    MMDT = getattr(mybir.dt, MM_DTYPE)
    AF = mybir.ActivationFunctionType

    # HARD CONSTRAINT on this toolchain: walrus allows at most ONE
    # sync-wait per instruction.  Tile elides a wait only if this engine
    # already waited a >= tick of that semaphore via an earlier
    # DATA-dependent instruction (manual sync edges / Drains don't
    # count).  The kernel threads tiny observer ops through every engine
    # so each real instruction needs <= 1 wait:
    #   PE  <- ACT ticks: tiny matmuls reading evicted tiles
    #   ACT <- DVE ticks: tiny Copy activations
    #   DVE <- PE ticks:  write-once PSUM "mailbox" stamped by PE
    #   SP  <- DVE ticks: tiny SBUF->DRAM observer DMA
    # Activations bounce DRAM -> per-k landing tile -> DVE copy ->
    # compute tile so a landing slot's accessor set is {one DMA, one DVE
    # copy} and its reuse wait collapses to one DMA-queue semaphore.
    # All DRAM tensors are chunk-major so per-chunk accesses are
    # disjoint regions (no conservative overlap waits).

    _patch_tail_drain()
    nc = bass.Bass()

    # ft|f2t merged chunk-major: ONE load DMA per chunk (issue cost on the
    # ACT HWDGE ring is ~0.7us per dma_start, so fewer+larger wins)
    fx = nc.declare_dram_parameter("fx", [NCHUNK, 128, 2 * KG * NF], MMDT,
                                   isOutput=False)
    wk = nc.declare_dram_parameter("wk", [C, D], MMDT, isOutput=False)
    wv1 = nc.declare_dram_parameter("wv1", [C, D], MMDT, isOutput=False)
    wv2 = nc.declare_dram_parameter("wv2", [D, D], MMDT, isOutput=False)
    # C-stage runs fp8e4 DoubleRow (2 contraction rows/cycle): the only
    # stage where fp8 quantization is safe (sigmoid damps the error and
    # the result only modulates the P*C term)
    FP8 = mybir.dt.float8e4
    wvc = nc.declare_dram_parameter("wvc", [D, D], FP8, isOutput=False)
    # [bn1s | bn1b | bn2s | bn2b | zeros] in one tensor -> one DMA
    bnv = nc.declare_dram_parameter("bnv", [128, 5 * MG + 128], FP32,
                                    isOutput=False)
    # one output tensor per chunk: DRAM dep tracking is tensor-granular,
    # so a shared output tensor would chain every out-DMA via WAW waits
    outs = [
        nc.declare_dram_parameter(f"out{j}", [D, NF], MMDT, isOutput=True)
        for j in range(NCHUNK)
    ]

    with tile.TileContext(nc) as tc:
        with ExitStack() as ctx:
            consts = ctx.enter_context(tc.tile_pool(name="consts", bufs=1))

            def load_weight(dram, dt=None):
                t = consts.tile([128, KG, D], dt or MMDT, tag=f"w_{dram.name}")
                nc.scalar.dma_start(
                    t[:], dram[:].rearrange("(k p) o -> p k o", p=128))
                return t

            wk_t = load_weight(wk)

            # weights + bnv issue on the ACT HWDGE ring; activation
            # chunks issue on the (otherwise idle) SP HWDGE ring so the
            # two rings fill in parallel at startup and the per-chunk
            # load issue never contends with ACT eviction work
            io_pool_early = ctx.enter_context(tc.tile_pool(name="ioe", bufs=2))
            early_loads = {}
            HLF = KG * NF // 2

            def early_load(j0, split=False):
                fxX0 = io_pool_early.tile([128, 2 * KG * NF], MMDT,
                                          tag="fxX", name=f"fxXe_{j0}")
                if split:
                    # chunk 0: ft-half first so the P stage can start
                    # ~1.2us before the f2t half lands
                    nc.scalar.dma_start(fxX0[:, 0 : KG * NF],
                                        fx[j0, :, 0 : KG * NF])
                    nc.scalar.dma_start(fxX0[:, KG * NF :],
                                        fx[j0, :, KG * NF :])
                else:
                    nc.scalar.dma_start(fxX0[:], fx[j0])
                early_loads[j0] = fxX0

            early_load(0, split=True)
            wv1_t = load_weight(wv1)
            wvc_t = load_weight(wvc, FP8)
            wv2_t = load_weight(wv2)

            bnv_t = consts.tile([128, 5 * MG + 128], FP32, tag="bnv")
            nc.scalar.dma_start(bnv_t[:], bnv[:])
            bn1s_t = bnv_t[:, 0 * MG : 1 * MG]
            bn1b_t = bnv_t[:, 1 * MG : 2 * MG]
            bn2s_t = bnv_t[:, 2 * MG : 3 * MG]
            bn2b_t = bnv_t[:, 3 * MG : 4 * MG]
            zero_t = bnv_t[:, 4 * MG : 5 * MG]
            ident_t = bnv_t[:, 5 * MG : 5 * MG + 128]

            early_load(1)

            # write-once scratch columns for observer writes
            scrA = consts.tile([128, 3 * NCHUNK + 16], FP32, tag="scrA")
            scrD = consts.tile([128, 2 * NCHUNK + 16], FP32, tag="scrD")

            io_pool = ctx.enter_context(tc.tile_pool(name="io", bufs=2))
            act_pool = ctx.enter_context(tc.tile_pool(name="acts", bufs=2))
            psA = ctx.enter_context(tc.tile_pool(name="psA", bufs=4, space="PSUM"))
            psB = ctx.enter_context(tc.tile_pool(name="psB", bufs=3, space="PSUM"))
            psM = ctx.enter_context(tc.tile_pool(name="psM", bufs=1, space="PSUM"))
            # weight-observer target (write-once columns)
            mb = psM.tile([128, 8], FP32, tag="mb")

            ALU = mybir.AluOpType

            def mm_stage(pool, w_t, src_t, dst_t, mo, func,
                         scale=1.0, bias=0.0, after=None, act_after=None,
                         dve_evict=False, src_base=0, fp8_dst=None):
                ps = pool.tile([128, NF], FP32, tag="ps")
                last = None
                for k in range(KG):
                    mm = nc.tensor.matmul(
                        ps[:],
                        lhsT=w_t[:, k, ds(mo * 128, 128)],
                        rhs=src_t[:, ds(src_base + k * NF, NF)],
                        start=(k == 0),
                        stop=(k == KG - 1),
                    )
                    last = mm.ins
                    if after is not None:
                        add_dep_helper(mm.ins, after, sync=False,
                                       reason="mm order")
                dst = dst_t[:, ds(mo * NF, NF)]
                if dve_evict:
                    # relu (+ bias) eviction on the DVE to offload the
                    # saturated ACT engine; BN scale is folded into the
                    # weights host-side
                    if isinstance(bias, float):
                        act = nc.vector.tensor_scalar_max(dst, ps[:], 0.0)
                    else:
                        act = nc.vector.tensor_scalar(
                            dst, ps[:], bias, 0.0, ALU.add, ALU.max)
                else:
                    act = nc.scalar.activation(
                        dst, ps[:], func, scale=scale, bias=bias)
                if act_after is not None:
                    add_dep_helper(act.ins, act_after, sync=False,
                                   reason="act order")
                if fp8_dst is not None:
                    # fp8 copy for the DoubleRow C matmul (P <= ~6 fits
                    # e4m3 unscaled); sourced from the bf16 SBUF tile, not
                    # PSUM — cheaper DVE read, and pure DVE-internal order
                    nc.vector.tensor_copy(
                        fp8_dst[:, ds(mo * NF, NF)], dst)
                return last, act.ins

            WVC_SCALE = 512.0  # host folds *512 into fp8 Wvc; descale here

            def dr_stage(pool, w_t, src_t, dst_t, mo, func,
                         scale=1.0, bias=0.0, after=None, act_after=None):
                # fp8 DoubleRow: 2 contraction-rows/cycle, so KG/2 matmuls
                ps = pool.tile([128, NF], FP32, tag="ps")
                last = None
                for k2 in range(KG // 2):
                    mm = nc.tensor.matmul(
                        ps[:],
                        lhsT=w_t[:, ds(2 * k2, 2), ds(mo * 128, 128)],
                        rhs=src_t[:, ds(2 * k2 * NF, 2 * NF)].rearrange(
                            "p (two n) -> p two n", two=2),
                        start=(k2 == 0),
                        stop=(k2 == KG // 2 - 1),
                        perf_mode=mybir.MatmulPerfMode.DoubleRow,
                    )
                    last = mm.ins
                    if after is not None:
                        add_dep_helper(mm.ins, after, sync=False,
                                       reason="mm order")
                act = nc.scalar.activation(
                    dst_t[:, ds(mo * NF, NF)], ps[:], func,
                    scale=scale, bias=bias)
                if act_after is not None:
                    add_dep_helper(act.ins, act_after, sync=False,
                                   reason="act order")
                return last, act.ins

            state = {}

            for j in range(NCHUNK):
                s = {}
                s2 = state.get(j - 2, {})
                s1 = state.get(j - 1, {})

                # ---- load: ONE merged ft|f2t DMA per chunk ----
                if j < 2:
                    fxX = early_loads[j]
                else:
                    fxX = io_pool_early.tile([128, 2 * KG * NF], MMDT,
                                             tag="fxX", name=f"fxX_{j}")
                    ld = nc.scalar.dma_start(fxX[:], fx[j])
                    if "ao_last" in s1:
                        add_dep_helper(ld.ins, s1["ao_last"], sync=False,
                                       reason="loads after prev ao")
                p_t = act_pool.tile([128, MG * NF], MMDT, tag="P")
                p8_t = act_pool.tile([128, MG * NF], FP8, tag="P8")
                h_t = act_pool.tile([128, MG * NF], MMDT, tag="H")
                v_t = act_pool.tile([128, MG * NF], MMDT, tag="V")
                c_t = act_pool.tile([128, MG * NF], MMDT, tag="Cc")
                o_t = io_pool.tile([128, MG * NF], MMDT, tag="O")

                # ---- ACT observers ----
                if j == 0:
                    ao_last = nc.scalar.activation(
                        scrA[0:1, 0:1], bnv_t[0:1, 0:1], AF.Copy).ins
                    # pre-warm ACT's DMA-lane wait history: the chunk-j
                    # load DMA's round-robin lane reuses a startup DMA's
                    # lane; without an earlier ACT-side wait on that lane
                    # the load would carry 2 sync waits (walrus max 1).
                    # Tiny data-dependent copies give ACT those waits.
                    prewarm = [
                        wk_t[0:1, 0, 0:1], wv1_t[0:1, 0, 0:1],
                        wv2_t[0:1, 0, 0:1],
                        early_loads[0][0:1, 0:1],
                        early_loads[0][0:1, KG * NF : KG * NF + 1],
                        early_loads[1][0:1, 0:1],
                    ]
                    for wi, ap in enumerate(prewarm):
                        aw = nc.scalar.activation(
                            scrA[0:1, ds(3 * NCHUNK + 8 + wi, 1)],
                            ap, AF.Copy).ins
                        add_dep_helper(aw, ao_last, sync=False,
                                       reason="prewarm order")
                        ao_last = aw
                else:
                    ao1 = nc.scalar.activation(
                        scrA[0:1, ds(3 * j, 1)],
                        s1["o_t"][0:1, 0:1], AF.Copy).ins
                    ao2 = nc.scalar.activation(
                        scrA[0:1, ds(3 * j + 1, 1)],
                        s1["v_t"][0:1, ds(3 * NF, 1)], AF.Copy).ins
                    add_dep_helper(ao2, ao1, sync=False, reason="act obs order")
                    ao_last = ao2
                s["ao_last"] = ao_last

                # ---- PE observers (interleaved with stages below) ----
                # j=0: one tiny matmul per weight, placed just before the
                # stage that uses it, so the P stream starts as soon as
                # wk+chunk0 land instead of after ALL weights.
                # j>0: obsA (ACT ticks thru C3 evict of j-1) before P;
                # obsB (ACT ticks thru V3 evict of j-1) after P, before H
                # — V3's eviction only completes ~0.7us after the last
                # j-1 matmul, so observing it at chunk start stalls PE.
                def weight_obs(i, w_t, after):
                    ob = nc.tensor.matmul(
                        mb[:, ds(2 * i, 2)],
                        lhsT=w_t[:, 0, 0:128],
                        rhs=w_t[:, 0, 0:2], start=True, stop=True).ins
                    if after is not None:
                        add_dep_helper(ob, after, sync=False,
                                       reason="wobs order")
                    return ob

                obs_c = obs_v = None  # j=0 weight observers, emitted below
                if j == 0:
                    obs_p = weight_obs(0, wk_t, None)
                else:
                    ps0 = psA.tile([128, NF], FP32, tag="ps")
                    obs_p = nc.tensor.matmul(
                        ps0[0:128, 0:2],
                        lhsT=s1["c_t"][:, ds(3 * NF, 128)],
                        rhs=s1["c_t"][:, ds(3 * NF, 2)],
                        start=True, stop=True).ins

                # ---- stages ----
                dve_anchor = s2.get("comb_add")
                if j == 0:
                    # one-time: DVE observes the bnv DMA queue before the
                    # bias-carrying evictions read it
                    dve_anchor = nc.vector.tensor_copy(
                        scrD[0:1, ds(2 * NCHUNK + 1, 1)],
                        bnv_t[0:1, 0:1]).ins
                p_last = None
                for mo in range(MG):
                    mm, act = mm_stage(psA, wk_t, fxX, p_t, mo, AF.Relu,
                                       after=obs_p, act_after=dve_anchor,
                                       dve_evict=True, fp8_dst=p8_t)
                    p_last = mm

                if j == 0:
                    obs_h = weight_obs(1, wv1_t, p_last)
                else:
                    ps0 = psB.tile([128, NF], FP32, tag="ps")
                    obs_h = nc.tensor.matmul(
                        ps0[0:128, 0:2],
                        lhsT=s1["v_t"][:, ds(3 * NF, 128)],
                        rhs=s1["v_t"][:, ds(3 * NF, 2)],
                        start=True, stop=True).ins
                    add_dep_helper(obs_h, p_last, sync=False,
                                   reason="obs order")

                h_last = None
                for mo in range(MG):
                    mm, act = mm_stage(
                        psB, wv1_t, fxX, h_t, mo, AF.Relu,
                        bias=bn1b_t[:, mo : mo + 1],
                        after=obs_h, act_after=dve_anchor,
                        dve_evict=True, src_base=KG * NF,
                    )
                    h_last = mm

                if j == 0:
                    obs_c = weight_obs(2, wvc_t, h_last)
                c_last = None
                for mo in range(MG):
                    mm, act = dr_stage(psA, wvc_t, p8_t, c_t, mo, AF.Sigmoid,
                                       scale=1.0 / WVC_SCALE,
                                       bias=zero_t[:, mo : mo + 1],
                                       after=obs_c, act_after=ao_last)
                    c_last = mm
                if j == 0:
                    obs_v = weight_obs(3, wv2_t, c_last)
                for mo in range(MG):
                    mm, act = mm_stage(
                        psB, wv2_t, h_t, v_t, mo, AF.Relu,
                        bias=bn2b_t[:, mo : mo + 1],
                        after=obs_v, act_after=ao_last,
                    )

                # ---- combine (DVE) ----
                do1 = None
                if "out_dma" in s2:
                    do1 = nc.vector.tensor_copy(
                        o_t[0:1, 0:1], scrD[0:1, ds(j - 1, 1)]).ins
                    if "comb_add" in s1:
                        add_dep_helper(do1, s1["comb_add"], sync=False,
                                       reason="do1 after j-1 combine")
                # combine runs uniform-MMDT (bf16: 2x DVE rate, half the
                # out-DMA bytes); bitcast only for the 4-byte float32r path
                p_f32 = p_t[:].bitcast(FP32) if MM_DTYPE == "float32r" else p_t[:]
                out_view = outs[j][:].rearrange("(m p) n -> p m n", p=128)
                if j == NCHUNK - 1:
                    # pipeline the final chunk per mo-group so the tail
                    # (evict -> combine -> out-DMA) overlaps instead of
                    # serializing after the last matmul
                    prev_op = do1
                    for mo in range(MG):
                        sl = ds(mo * NF, NF)
                        oc = nc.vector.tensor_copy(
                            scrD[0:1, ds(2 * NCHUNK + 2 + 2 * mo, 1)],
                            c_t[0:1, ds(mo * NF, 1)]).ins
                        if prev_op is not None:
                            add_dep_helper(oc, prev_op, sync=False,
                                           reason="tail order")
                        cm = nc.vector.tensor_mul(
                            o_t[:, sl], p_f32[:, sl], c_t[:, sl])
                        add_dep_helper(cm.ins, oc, sync=False,
                                       reason="tail order")
                        ov = nc.vector.tensor_copy(
                            scrD[0:1, ds(2 * NCHUNK + 3 + 2 * mo, 1)],
                            v_t[0:1, ds(mo * NF, 1)]).ins
                        add_dep_helper(ov, cm.ins, sync=False,
                                       reason="tail order")
                        ca = nc.vector.tensor_add(
                            o_t[:, sl], o_t[:, sl], v_t[:, sl])
                        add_dep_helper(ca.ins, ov, sync=False,
                                       reason="tail order")
                        ao3 = nc.scalar.activation(
                            scrA[0:1, ds(3 * j + 2, 1)] if mo == 0 else
                            scrA[0:1, ds(3 * NCHUNK + mo, 1)],
                            o_t[0:1, ds(mo * NF + 1, 1)], AF.Copy).ins
                        od = nc.scalar.dma_start(out_view[:, mo, :],
                                                 o_t[:, sl])
                        add_dep_helper(od.ins, ao3, sync=False,
                                       reason="od after ao3")
                        prev_op = ca.ins
                    s["comb_add"] = prev_op
                    s["out_dma"] = od.ins
                else:
                    # DVE observes C3's ACT tick, multiplies, then observes
                    # V3's tick and adds — the mul doesn't idle until the
                    # V evictions land
                    do2a = nc.vector.tensor_copy(
                        scrD[0:1, ds(j, 1)], c_t[0:1, ds(3 * NF, 1)]).ins
                    if do1 is not None:
                        add_dep_helper(do2a, do1, sync=False,
                                       reason="dve order")
                    cm = nc.vector.tensor_mul(o_t[:], p_f32, c_t[:])
                    add_dep_helper(cm.ins, do2a, sync=False,
                                   reason="mul order")
                    do2b = nc.vector.tensor_copy(
                        scrD[0:1, ds(NCHUNK + 1 + j, 1)],
                        v_t[0:1, ds(3 * NF, 1)]).ins
                    add_dep_helper(do2b, cm.ins, sync=False,
                                   reason="dve order")
                    ca = nc.vector.tensor_add(o_t[:], o_t[:], v_t[:])
                    add_dep_helper(ca.ins, do2b, sync=False,
                                   reason="add order")
                    s["comb_add"] = ca.ins

                    # ACT observes this chunk's combine so the (ACT-issued)
                    # out-DMA needs no fresh DVE wait of its own
                    ao3 = nc.scalar.activation(
                        scrA[0:1, ds(3 * j + 2, 1)], o_t[0:1, 1:2],
                        AF.Copy).ins
                    od = nc.scalar.dma_start(
                        out_view,
                        o_t[:].rearrange("p (m n) -> p m n", m=MG),
                    )
                    add_dep_helper(od.ins, ao3, sync=False,
                                   reason="od after ao3")
                    s["out_dma"] = od.ins
                s["c_t"] = c_t
                s["v_t"] = v_t
                s["o_t"] = o_t
                state[j] = s

    return nc


def _np_mm_dtype():
    if MM_DTYPE == "bfloat16":
        import ml_dtypes

        return np.dtype(ml_dtypes.bfloat16)
    return np.dtype(np.float32)


def _prep_inputs(inputs):
    mmdt = _np_mm_dtype()
    f = np.asarray(inputs["features"], np.float32)
    f2 = np.asarray(inputs["features2"], np.float32)

    def wT(name):
        return np.ascontiguousarray(np.asarray(inputs[name], np.float32).T)

    wk_h, wv1_h, wv2_h, wvc_h = wT("Wk"), wT("Wv1"), wT("Wv2"), wT("Wvc")

    def bn_inv(pre):
        g = np.asarray(inputs[f"{pre}_gamma"], np.float32)
        v = np.asarray(inputs[f"{pre}_var"], np.float32)
        return g / np.sqrt(v + BN_EPS)

    # fold the BN scale into the weight columns (bias stays separate)
    wv1_h = np.ascontiguousarray(wv1_h * bn_inv("bn1")[None, :])
    wv2_h = np.ascontiguousarray(wv2_h * bn_inv("bn2")[None, :])

    def bn_fold(pre):
        g = np.asarray(inputs[f"{pre}_gamma"], np.float32)
        b = np.asarray(inputs[f"{pre}_beta"], np.float32)
        m = np.asarray(inputs[f"{pre}_mean"], np.float32)
        v = np.asarray(inputs[f"{pre}_var"], np.float32)
        inv = g / np.sqrt(v + BN_EPS)
        shift = b - m * inv
        to_tile = lambda x: np.ascontiguousarray(x.reshape(MG, 128).T)
        return to_tile(inv), to_tile(shift)

    bn1s_h, bn1b_h = bn_fold("bn1")
    bn2s_h, bn2b_h = bn_fold("bn2")
    bnv_h = np.ascontiguousarray(np.concatenate(
        [bn1s_h, bn1b_h, bn2s_h, bn2b_h,
         np.zeros((128, MG), np.float32),
         np.zeros((128, 128), np.float32)],
        axis=1,
    ))

    import ml_dtypes

    shared = {
        "wk": wk_h.astype(mmdt), "wv1": wv1_h.astype(mmdt),
        "wv2": wv2_h.astype(mmdt),
        # C-stage weights: fp8e4 scaled by 512 (descaled in the sigmoid);
        # |Wvc|*512 tops out ~120, well under e4m3's 240 max
        "wvc": np.ascontiguousarray(
            (wvc_h * 512.0).astype(ml_dtypes.float8_e4m3)),
        "bnv": bnv_h,
    }

    def chunked_T(x):  # [NS, C] rows -> [NCHUNK, 128, KG*NF] k-major
        t = x.T.astype(mmdt)  # [C, NS]; c = k*128 + p
        a = t.reshape(KG, 128, NCHUNK, NF)
        return np.ascontiguousarray(
            a.transpose(2, 1, 0, 3).reshape(NCHUNK, 128, KG * NF))

    in_maps = []
    for i in range(N_CORES):
        rows = slice(i * NS, (i + 1) * NS)
        in_maps.append({
            "fx": np.ascontiguousarray(np.concatenate(
                [chunked_T(f[rows]), chunked_T(f2[rows])], axis=2)),
            **shared,
        })
    return in_maps


def _gather_out(res_map):
    """[NCHUNK x [D, NF]] per-core outputs -> [NS, D] rows."""
    chunks = [np.asarray(res_map[f"out{j}"]) for j in range(NCHUNK)]
    stacked = np.stack(chunks, axis=0)          # [NCHUNK, D, NF]
    return stacked.transpose(0, 2, 1).reshape(NS, D)


def run(inputs, trace=False):
    from concourse.bass_utils import run_bass_kernel_spmd

    if "nc" not in _CACHE:
        _CACHE["nc"] = _build_program()
    nc = _CACHE["nc"]

    in_maps = _prep_inputs(inputs)
    res = run_bass_kernel_spmd(
        nc, in_maps, list(range(N_CORES)), trace=trace
    )
    full = np.concatenate(
        [_gather_out(r) for r in res.results], axis=0
    ).astype(np.float32)
    return full, res


def kernel(**inputs) -> np.ndarray:
    out, _ = run(inputs, trace=False)
    return out

